# revision 5
# baseline (speedup 1.0000x reference)
"""Trainium2 Bass kernel for nn_CPA_learnStep.

Problem: per batch b (256 total): scores = f1 @ f2^T / sqrt(C); pred = softmax;
P = 2D cumsum; block[i,j,a,bb] = (P[i,j]-P[i,bb]-P[a,j]+P[a,bb]) / sqrt(max(1,(i-a)(j-bb)))
masked to a<i, bb<j; 15-step max-plus DP D_k[i,j] = max_{a,bb}(D_{k-1}[a,bb]+block);
outputs loss = -max_k D_k[15,15], step = argmax_k.

Mapping (per core, 32 batches = 4 octets of 8):
 - partition p = (bo, i) with bo = batch-in-octet (8), i (16).
 - scores: cross-batch 128x128 fp32 matmuls over 16 C-chunks, diagonal 16x16
   blocks extracted.
 - softmax + exact 2D cumsum (PE transposes + DVE segmented scans) -> P [128,16].
 - H[p,a,x] = (P[p,x] - P[(bo,a),x]) * u[i,a]; H2t = H + 1e33*[a>=i].
   (cross-partition P replicate via DRAM round-trip broadcast DMA)
 - blockf0[p, j-region (a, bb<j)] = (H[:,a,j] - H2t[:,a,bb]) * v[j,bb]
 - recurrence step k: per octet: replicate D via DRAM round trip; per j in [k,15]:
   tensor_tensor_reduce(add, max) over a in [k-1,16), bb in [k-1,j) -> D[:, j].
 - finres taps D[:,15] per step; host does -max/argmax (exact fp32, matching
   reference tie-breaking).
"""
import math
from contextlib import ExitStack

import numpy as np

T = 16
C = 2048
B = 256
NCORES = 8
BPC = B // NCORES          # 32 batches per core
NOCT = BPC // 8            # 4 octets
P = 128
NEGINIT = -1e38
QABIG = 1e33

_PROG = None  # cached (nc, names)


def _consts():
    i = np.arange(T)
    # u[i,a] = 1/sqrt(max(1, i-a)); v identical by symmetry
    u2 = (1.0 / np.sqrt(np.maximum(1.0, i[:, None] - i[None, :]))).astype(np.float32)
    qa2 = np.where(i[None, :] >= i[:, None], np.float32(QABIG), np.float32(0.0))
    ii = np.tile(i, 8)  # i index per partition
    u256 = np.broadcast_to(u2[ii][:, :, None], (P, T, T)).astype(np.float32)   # [p, a, x]
    qa256 = np.broadcast_to(qa2[ii][:, :, None], (P, T, T)).astype(np.float32)
    vc = np.broadcast_to(u2[None, :, :], (P, T, T)).astype(np.float32)         # [p, j, bb]
    den = ((ii[:, None] + 1.0) * (i[None, :] + 1.0)).astype(np.float32)        # [p, j]
    invden = (np.float32(1.0) / den).astype(np.float32)
    return u256, np.ascontiguousarray(qa256), np.ascontiguousarray(vc), invden




def _build_program():
    import concourse.bacc as bacc
    import concourse.tile as tile
    from concourse import mybir

    F32 = mybir.dt.float32
    ALU = mybir.AluOpType
    ACTF = mybir.ActivationFunctionType
    AX = mybir.AxisListType
    INV_SQRT_C = 1.0 / math.sqrt(C)

    nc = bacc.Bacc("TRN2", target_bir_lowering=False, debug=False,
                   num_devices=NCORES)

    f1t_d = nc.dram_tensor("f1t", [C, 4 * P], F32, kind="ExternalInput").ap()
    f2t_d = nc.dram_tensor("f2t", [C, 4 * P], F32, kind="ExternalInput").ap()
    fin_d = nc.dram_tensor("finres", [P, NOCT * T], F32, kind="ExternalOutput").ap()

    ddram = [nc.dram_tensor(f"dbounce{o}", [8, T, T], F32, kind="Internal").ap()
             for o in range(NOCT)]
    pdram = [nc.dram_tensor(f"pbounce{o}", [8, T, T], F32, kind="Internal").ap()
             for o in range(NOCT)]

    u256_np, qa256_np, vc_np, invden_np = _consts()
    u256_d = nc.inline_tensor(u256_np, name="u256")
    qa256_d = nc.inline_tensor(qa256_np, name="qa256")
    vc_d = nc.inline_tensor(vc_np, name="vc")
    invden_d = nc.inline_tensor(invden_np, name="invden")
    ident_d = nc.inline_tensor(np.eye(P, dtype=np.float32), name="ident")
    negfill_np = np.full((P, (T - 1) * T * T), np.float32(NEGINIT), np.float32)
    negfill_d = nc.inline_tensor(negfill_np, name="negfill")

    with tile.TileContext(nc) as tc, ExitStack() as ctx:
        const_pool = ctx.enter_context(tc.tile_pool(name="consts", bufs=1))
        feat_pool = ctx.enter_context(tc.tile_pool(name="feat", bufs=3))
        main_pool = ctx.enter_context(tc.tile_pool(name="main", bufs=1))
        tmp_pool = ctx.enter_context(tc.tile_pool(name="tmp", bufs=2))
        drep_pool = ctx.enter_context(tc.tile_pool(name="drep", bufs=2))
        psum_pool = ctx.enter_context(tc.tile_pool(name="psum", bufs=1, space="PSUM"))
        psum_tmp = ctx.enter_context(tc.tile_pool(name="psumtmp", bufs=2, space="PSUM"))

        u256 = const_pool.tile([P, T, T], F32)
        qa256 = const_pool.tile([P, T, T], F32)
        vc = const_pool.tile([P, T, T], F32)
        invden = const_pool.tile([P, T], F32)
        ident = const_pool.tile([P, P], F32)
        for t, d in [(u256, u256_d), (qa256, qa256_d), (vc, vc_d),
                     (invden, invden_d), (ident, ident_d)]:
            nc.sync.dma_start(t[:], d.ap())

        # ---------------- scores: accumulate cross matmuls ----------------
        NCHUNK = C // P  # 16
        sc_psum = [psum_pool.tile([P, P], F32, tag=f"scps{o}", name=f"scps{o}") for o in range(NOCT)]
        for ch in range(NCHUNK):
            f1sb = feat_pool.tile([P, 4 * P], F32, tag="f1sb")
            f2sb = feat_pool.tile([P, 4 * P], F32, tag="f2sb")
            nc.sync.dma_start(f1sb[:], f1t_d[ch * P:(ch + 1) * P, :])
            nc.sync.dma_start(f2sb[:], f2t_d[ch * P:(ch + 1) * P, :])
            for o in range(NOCT):
                nc.tensor.matmul(sc_psum[o][:],
                                 f1sb[:, o * P:(o + 1) * P],
                                 f2sb[:, o * P:(o + 1) * P],
                                 start=(ch == 0), stop=(ch == NCHUNK - 1))

        # per-octet state tiles
        D_t = [main_pool.tile([P, T], F32, tag=f"D{o}", name=f"D{o}") for o in range(NOCT)]
        H_t = [main_pool.tile([P, T, T], F32, tag=f"H{o}", name=f"H{o}") for o in range(NOCT)]
        H2_t = [main_pool.tile([P, T, T], F32, tag=f"H2{o}", name=f"H2{o}") for o in range(NOCT)]
        blockf = [main_pool.tile([P, T - 1, T, T], F32, tag=f"bf{o}", name=f"bf{o}")
                  for o in range(NOCT)]
        tmp4 = main_pool.tile([P, T - 1, T, T], F32)
        for o in range(NOCT):
            nc.sync.dma_start(
                blockf[o][:].rearrange("p j a b -> p (j a b)"),
                negfill_d.ap())
        finstage = main_pool.tile([P, NOCT * T], F32)

        # ---------------- per octet: softmax, cumsums, H, blockf ----------------
        for o in range(NOCT):
            scfull = tmp_pool.tile([P, P], F32, tag="scfull")
            nc.scalar.copy(scfull[:], sc_psum[o][:])
            ssb = tmp_pool.tile([P, T], F32, tag="ssb")
            for bo in range(8):
                sl = slice(bo * T, (bo + 1) * T)
                nc.sync.dma_start(ssb[sl, :], scfull[sl, bo * T:(bo + 1) * T])
            rowmax = tmp_pool.tile([P, 1], F32, tag="rowmax")
            nc.vector.tensor_reduce(rowmax[:], ssb[:], AX.X, ALU.max)
            negmaxs = tmp_pool.tile([P, 1], F32, tag="negmaxs")
            nc.vector.tensor_scalar_mul(negmaxs[:], rowmax[:], -INV_SQRT_C)
            e_t = tmp_pool.tile([P, T], F32, tag="e_t")
            rowsum = tmp_pool.tile([P, 1], F32, tag="rowsum")
            nc.scalar.activation(e_t[:], ssb[:], ACTF.Exp, bias=negmaxs[:],
                                 scale=INV_SQRT_C, accum_out=rowsum[:])
            recip = tmp_pool.tile([P, 1], F32, tag="recip")
            nc.vector.reciprocal(recip[:], rowsum[:])
            pred = tmp_pool.tile([P, T], F32, tag="pred")
            nc.vector.tensor_scalar_mul(pred[:], e_t[:], recip[:])

            # cumsum over i (partition direction): transpose, segmented scans,
            # transpose back; then cumsum over j (free) -> P_sb
            predT_ps = psum_tmp.tile([T, P], F32, tag="predT")
            nc.tensor.transpose(predT_ps[:], pred[:], ident[:])
            predT = tmp_pool.tile([T, P], F32, tag="predTsb")
            nc.scalar.copy(predT[:], predT_ps[:])
            predTc = tmp_pool.tile([T, P], F32, tag="predTc")
            for bo in range(8):
                sl = slice(bo * T, (bo + 1) * T)
                nc.vector.tensor_tensor_scan(
                    predTc[:, sl], predT[:, sl], predT[:, sl], 0.0,
                    op0=ALU.add, op1=ALU.bypass)
            predC_ps = psum_tmp.tile([P, T], F32, tag="predC")
            nc.tensor.transpose(predC_ps[:], predTc[:], ident[0:T, 0:T])
            P_sb = tmp_pool.tile([P, T], F32, tag="P_sb")
            nc.vector.tensor_tensor_scan(
                P_sb[:], predC_ps[:], invden[:], 0.0,
                op0=ALU.add, op1=ALU.bypass)

            # D0 + tap
            nc.vector.tensor_tensor(D_t[o][:], P_sb[:], invden[:], ALU.mult)
            nc.scalar.copy(finstage[:, o * T:o * T + 1], D_t[o][:, T - 1:T])

            # replicate P across i-partitions via DRAM round trip
            nc.sync.dma_start(pdram[o].rearrange("b i j -> (b i) j"), P_sb[:])
            Prep = tmp_pool.tile([P, T, T], F32, tag="prep")
            nc.sync.dma_start(Prep[:],
                              pdram[o].unsqueeze(1).to_broadcast((8, T, T, T)))

            # H = (P_bcast - Prep) * u256 ; H2t = H + qa256
            nc.vector.tensor_tensor(
                H_t[o][:], P_sb[:].unsqueeze(1).to_broadcast((P, T, T)),
                Prep[:], ALU.subtract)
            nc.vector.tensor_tensor(H_t[o][:], H_t[o][:], u256[:], ALU.mult)
            nc.vector.tensor_tensor(H2_t[o][:], H_t[o][:], qa256[:], ALU.add)

            # blockf build, per j (bb >= j cells stay at NEGINIT from the fill)
            for j in range(1, T):
                reg = blockf[o][:, j - 1, :, 0:j]
                nc.vector.tensor_tensor(
                    reg, H_t[o][:, :, j].unsqueeze(2).to_broadcast((P, T, j)),
                    H2_t[o][:, :, 0:j], ALU.subtract)
                nc.vector.tensor_tensor(
                    reg, reg,
                    vc[:, j, 0:j].unsqueeze(1).to_broadcast((P, T, j)),
                    ALU.mult)

        # ---------------- recurrence ----------------
        for k in range(1, T):
            dreps = []
            for o in range(NOCT):
                dd = ddram[o]
                nc.sync.dma_start(dd.rearrange("b i j -> (b i) j"), D_t[o][:])
                Drep = drep_pool.tile([P, T, T], F32, tag=f"drep{o}", name=f"drep{o}")
                nc.sync.dma_start(Drep[:],
                                  dd.unsqueeze(1).to_broadcast((8, T, T, T)))
                dreps.append(Drep)
            nj = T - k      # j in [k, 15] -> blockf j-axis [k-1, 15)
            na = T - (k - 1)
            for o in range(NOCT):
                tsl = tmp4[:, k - 1:, k - 1:, k - 1:]
                nc.vector.tensor_tensor(
                    tsl, blockf[o][:, k - 1:, k - 1:, k - 1:],
                    dreps[o][:, k - 1:, k - 1:].unsqueeze(1).to_broadcast(
                        (P, nj, na, na)),
                    ALU.add)
                nc.vector.tensor_reduce(
                    D_t[o][:, k:T], tsl, AX.XY, ALU.max)
                nc.scalar.copy(finstage[:, o * T + k:o * T + k + 1],
                               D_t[o][:, T - 1:T])

        nc.sync.dma_start(fin_d, finstage[:])

    nc.compile()
    return nc


def _get_prog():
    global _PROG
    if _PROG is None:
        _PROG = _build_program()
    return _PROG


def _prep_core_inputs(f1, f2, core):
    lo = core * BPC
    f1c = f1[lo:lo + BPC].reshape(NOCT, 8, T, C)
    f2c = f2[lo:lo + BPC].reshape(NOCT, 8, T, C)
    # [C, (o, bo, i)]
    f1t = np.ascontiguousarray(f1c.transpose(3, 0, 1, 2).reshape(C, 4 * P))
    f2t = np.ascontiguousarray(f2c.transpose(3, 0, 1, 2).reshape(C, 4 * P))
    return {"f1t": f1t, "f2t": f2t}


def kernel(seq_features1, seq_features2):
    from concourse.bass_utils import run_bass_kernel_spmd

    f1 = np.ascontiguousarray(np.asarray(seq_features1, dtype=np.float32))
    f2 = np.ascontiguousarray(np.asarray(seq_features2, dtype=np.float32))
    nc = _get_prog()

    in_maps = [_prep_core_inputs(f1, f2, core) for core in range(NCORES)]
    res = run_bass_kernel_spmd(nc, in_maps, core_ids=list(range(NCORES)))

    finres = np.empty((B, T), np.float32)
    for core in range(NCORES):
        out = np.asarray(res.results[core]["finres"])  # [128, 64]
        for o in range(NOCT):
            for bo in range(8):
                b = core * BPC + o * 8 + bo
                finres[b] = out[bo * T + (T - 1), o * T:(o + 1) * T]

    loss_step = (-finres.max(axis=-1)).astype(np.float32)
    step_num = finres.argmax(axis=-1).astype(np.int32)
    return loss_step, step_num
